# revision 1
# baseline (speedup 1.0000x reference)
"""NodeContrastiveLoss on 8 Trainium2 NeuronCores (Bass/Tile).

loss = mean_i[ -(z1n_i . z2n_i)/tau
               + log( sum_j exp((z1n_i . z2n_j)/tau)
                    + sum_{j!=i} exp((z1n_i . z1n_j)/tau) ) ]

Sharding: z1 query rows split 8 ways (2048 rows/core); every core builds the
full normalized key matrices z1n^T, z2n^T in SBUF (bf16) and computes its row
block of the similarity logits with PE matmuls, fusing exp + row-sum on the
Scalar engine (activation accum_out). Since |sim/tau| <= 1/tau ~ 14.3, plain
sum-of-exp in f32 is a stable logsumexp (no max pass). The z1-z1 diagonal is
removed by subtracting exp(||z1n_i||^2/tau) computed from the same bf16
values the PE consumes.

The Scalar engine's exp stream is the bound (~2.2us per 2048-key chunk, 256
chunks). Key prep is kept off it: GpSimd squares + DVE reduce/scale for the
row norms, and batched DMA-xbar transposes (one 32-tile dma_start_transpose
per 4096-row group, ~2.5us on the DMA stream) build keysT. PE does nothing
but matmuls into a double-buffered 2x4-bank PSUM pool feeding ACT.
"""

import os
import numpy as np

N, D = 16384, 128
TAU = 0.07
NCORES = 8
NQ = N // NCORES          # 2048 query rows per core
P = 128
QT = NQ // P              # 16 query tiles per core
GROUP = 32                # row tiles per staging group (4096 rows)
CHUNK = 2048              # keys per exp/accumulate chunk (4 PSUM banks)
SUB = 512                 # matmul moving free dim
NGRP = N // (GROUP * P)   # 4 groups per key matrix
NCHUNKS = 2 * N // CHUNK  # 16 global chunks (z2 then z1)

_CACHE = {}


def _split_excess_waits(nc, mybir):
    """walrus in this env supports 1 sync-wait per instruction (2 for
    EventSemaphore); move excess waits onto injected same-engine NoOps."""
    n = 0
    for f in nc.m.functions:
        for bb in f.blocks:
            new_insts = None
            for idx, inst in enumerate(bb.instructions):
                si = getattr(inst, "sync_info", None)
                waits = list(si.on_wait) if si is not None and si.on_wait else []
                cap = 2 if getattr(inst, "opcode", None) == "EventSemaphore" else 1
                if len(waits) <= cap:
                    if new_insts is not None:
                        new_insts.append(inst)
                    continue
                if new_insts is None:
                    new_insts = list(bb.instructions[:idx])
                keep, excess = waits[-cap:], waits[:-cap]
                for w in excess:
                    n += 1
                    nop = mybir.InstNoOp(name=f"I-wsplit-{n}-{inst.name}", ins=[], outs=[])
                    nop.engine = inst.engine
                    nop.sync_info = mybir.SyncInfo(on_wait=[w], on_update=[])
                    new_insts.append(nop)
                si.on_wait = keep
                new_insts.append(inst)
            if new_insts is not None:
                bb.instructions = new_insts
    return n


def _build_nc():
    from contextlib import ExitStack

    import concourse.bass as bass
    import concourse.tile as tile
    from concourse import mybir

    F32 = mybir.dt.float32
    BF16 = mybir.dt.bfloat16
    AF = mybir.ActivationFunctionType
    ALU = mybir.AluOpType
    AX = mybir.AxisListType

    nc = bass.Bass("TRN2", target_bir_lowering=False, debug=False)
    z1 = nc.declare_dram_parameter("z1", [N, D], F32, isOutput=False).ap()
    z2 = nc.declare_dram_parameter("z2", [N, D], F32, isOutput=False).ap()
    z1q = nc.declare_dram_parameter("z1q", [NQ, D], F32, isOutput=False).ap()
    z2q = nc.declare_dram_parameter("z2q", [NQ, D], F32, isOutput=False).ap()
    out = nc.declare_dram_parameter("out", [P, QT], F32, isOutput=True).ap()

    with tile.TileContext(nc) as tc, ExitStack() as ctx:
        persist = ctx.enter_context(tc.tile_pool(name="persist", bufs=1))
        stage_p = ctx.enter_context(tc.tile_pool(name="stage", bufs=2))
        norm_p = ctx.enter_context(tc.tile_pool(name="norms", bufs=2))
        nbg_p = ctx.enter_context(tc.tile_pool(name="nbg", bufs=2))
        work_p = ctx.enter_context(tc.tile_pool(name="work", bufs=4))
        dum_p = ctx.enter_context(tc.tile_pool(name="dum", bufs=2))
        ps_p = ctx.enter_context(tc.tile_pool(name="ps", bufs=2, space="PSUM"))

        z1T = persist.tile([P, N], BF16, tag="z1T")
        z2T = persist.tile([P, N], BF16, tag="z2T")
        z1qT = persist.tile([P, NQ], BF16, tag="z1qT")
        z2qn = persist.tile([P, NQ], F32, tag="z2qn")
        pos_raw = persist.tile([P, QT], F32, tag="pos")
        d_raw = persist.tile([P, QT], F32, tag="draw")
        S_raw = persist.tile([P, QT], F32, tag="sraw")
        part = persist.tile([P, QT * NCHUNKS], F32, tag="part")

        def rsqrt_newton(ssq, ntiles):
            """r = 1/sqrt(ssq) elementwise over [P, ntiles]; ACT sqrt seed
            + DVE reciprocal + one DVE Newton step."""
            r0 = norm_p.tile([P, GROUP], F32, tag="r0")
            t1 = norm_p.tile([P, GROUP], F32, tag="t1")
            # rsqrt seed via exp(-0.5*ln(s)): stays in the natural_log_exp
            # ACT table set (no table switches against the main Exp stream)
            nc.scalar.activation(r0[:, :ntiles], ssq[:, :ntiles], AF.Ln)
            nc.scalar.activation(r0[:, :ntiles], r0[:, :ntiles], AF.Exp,
                                 bias=0.0, scale=-0.5)
            nc.vector.tensor_mul(t1[:, :ntiles], r0[:, :ntiles], r0[:, :ntiles])
            nc.vector.tensor_mul(t1[:, :ntiles], t1[:, :ntiles], ssq[:, :ntiles])
            nc.vector.tensor_scalar(
                out=t1[:, :ntiles], in0=t1[:, :ntiles],
                scalar1=-0.5, scalar2=1.5, op0=ALU.mult, op1=ALU.add,
            )
            nc.vector.tensor_mul(r0[:, :ntiles], r0[:, :ntiles], t1[:, :ntiles])
            return r0

        def load_group(src, row0, ntiles):
            """DMA ntiles row tiles to staging; compute 1/norm per row
            (GpSimd squares, DVE reduces — keeps ACT free)."""
            stage = stage_p.tile([P, GROUP, P], F32, tag="stage")
            nc.sync.dma_start(
                out=stage[:, :ntiles, :],
                in_=src[row0:row0 + ntiles * P, :].rearrange("(t p) d -> p t d", p=P),
            )
            ssq = norm_p.tile([P, GROUP], F32, tag="ssq")
            for t in range(ntiles):
                sq = work_p.tile([P, P], F32, tag="sq")
                # fused square+row-sum in one DVE op: out=(in0 bypass s)*in1
                nc.vector.scalar_tensor_tensor(
                    out=sq[:, :], in0=stage[:, t, :], scalar=1.0,
                    in1=stage[:, t, :], op0=ALU.bypass, op1=ALU.mult,
                    accum_out=ssq[:, t:t + 1],
                )
            return stage, rsqrt_newton(ssq, ntiles)

        def normalize_group(stage, r, ntiles):
            """DVE per-tile scale+cast into one contiguous bf16 buffer."""
            nbg = nbg_p.tile([P, GROUP * P], BF16, tag="nbg")
            for t in range(ntiles):
                nc.vector.tensor_scalar_mul(
                    nbg[:, t * P:(t + 1) * P], stage[:, t, :], r[:, t:t + 1])
            return nbg

        def transpose_group(nbg, dst_T, col0, ntiles):
            """one batched DMA-xbar transpose: [P, ntiles*P] -> ntiles tiles."""
            dst3 = dst_T[:, col0:col0 + ntiles * P].rearrange(
                "p (t d) -> p t d", d=P)
            nc.sync.dma_start_transpose(dst3, nbg[:, :ntiles * P])

        def exp_unit(q, ck, keysT, koff):
            """4 matmuls filling a 4-bank PSUM slot + fused exp/row-sum."""
            ps = ps_p.tile([P, CHUNK], F32, tag="ps")
            kxm = z1qT[:, q * P:(q + 1) * P]
            for j in range(4):
                nc.tensor.matmul(
                    ps[:, j * SUB:(j + 1) * SUB],
                    lhsT=kxm,
                    rhs=keysT[:, koff + j * SUB: koff + (j + 1) * SUB],
                    start=True, stop=True,
                )
            # exp written back over the PSUM chunk in place (elementwise 1:1;
            # only the accum_out row-sum is consumed) — ScE->PSUM is the
            # faster ACT dst and it avoids an SBUF dummy buffer
            nc.scalar.activation(
                ps[:, :], ps[:, :], AF.Exp, bias=0.0, scale=1.0 / TAU,
                accum_out=part[:, q * NCHUNKS + ck: q * NCHUNKS + ck + 1],
            )

        # ---------------- prologue: only what the exp stream needs ----------
        # (z1q normalized bf16 + its transpose; everything else — z2q chain,
        # pos, d — is deferred under the exp stream)
        z1qn = persist.tile([P, NQ], BF16, tag="z1qn")
        z1qnf = persist.tile([P, NQ], F32, tag="z1qnf")

        stage, r = load_group(z1q, 0, QT)
        for t in range(QT):
            nc.vector.tensor_scalar_mul(
                z1qn[:, t * P:(t + 1) * P], stage[:, t, :], r[:, t:t + 1])
        transpose_group(z1qn, z1qT, 0, QT)
        # f32 normalized z1q rows, for pos (deferred consumer)
        for t in range(QT):
            nc.vector.tensor_scalar_mul(
                z1qnf[:, t * P:(t + 1) * P], stage[:, t, :], r[:, t:t + 1])

        def deferred_qprep():
            """z2q chain + pos + d: runs in engine slack under the exps."""
            stg, rq = load_group(z2q, 0, QT)
            for t in range(QT):
                nc.vector.tensor_scalar_mul(
                    z2qn[:, t * P:(t + 1) * P], stg[:, t, :], rq[:, t:t + 1])
            for t in range(QT):
                # d_raw[:, t] = sum_d bf16(z1n)^2 (matches the PE diag dot)
                sq = work_p.tile([P, P], F32, tag="sq")
                nc.gpsimd.tensor_mul(sq[:, :], z1qn[:, t * P:(t + 1) * P],
                                     z1qn[:, t * P:(t + 1) * P])
                nc.vector.tensor_reduce(
                    out=d_raw[:, t:t + 1], in_=sq[:, :], axis=AX.X, op=ALU.add)
                # pos_raw[:, t] = sum_d z1n * z2n (f32)
                mb = work_p.tile([P, P], F32, tag="mb")
                nc.gpsimd.tensor_mul(mb[:, :], z1qnf[:, t * P:(t + 1) * P],
                                     z2qn[:, t * P:(t + 1) * P])
                nc.vector.tensor_reduce(
                    out=pos_raw[:, t:t + 1], in_=mb[:, :], axis=AX.X, op=ALU.add)

        # ---------------- steady state ----------------
        # groups of 32 row tiles; z2 -> chunks 0..7, z1 -> chunks 8..15.
        groups = []
        for m, (src, dst_T) in enumerate(((z2, z2T), (z1, z1T))):
            for g in range(NGRP):
                groups.append((src, dst_T, g, m * (NGRP * 2) + g * 2))

        # prime group 0
        src0, dstT0, g0, _ = groups[0]
        stage_cur, r_cur = load_group(src0, g0 * GROUP * P, GROUP)
        nbg = normalize_group(stage_cur, r_cur, GROUP)
        transpose_group(nbg, dstT0, g0 * GROUP * P, GROUP)

        for gi, (src, dst_T, g, ckbase) in enumerate(groups):
            # prep the NEXT group (overlaps this group's exp stream)
            if gi + 1 < len(groups):
                nsrc, ndst, ng, _ = groups[gi + 1]
                stage_nxt, r_nxt = load_group(nsrc, ng * GROUP * P, GROUP)
                nbg_nxt = normalize_group(stage_nxt, r_nxt, GROUP)
                transpose_group(nbg_nxt, ndst, ng * GROUP * P, GROUP)
            if gi == 0:
                # fill engine slack under group 0's exps
                deferred_qprep()

            # 32 exp units for this group's two 2048-key chunks
            for half in range(2):
                ck = ckbase + half
                koff = (g * GROUP + half * (GROUP // 2)) * P
                for q in range(QT):
                    exp_unit(q, ck, dst_T, koff)

        # ---------------- epilogue: per-row losses ----------------
        for q in range(QT):
            nc.vector.tensor_reduce(
                out=S_raw[:, q:q + 1],
                in_=part[:, q * NCHUNKS:(q + 1) * NCHUNKS],
                axis=AX.X, op=ALU.add,
            )
        exp_d = work_p.tile([P, QT], F32, tag="expd")
        nc.scalar.activation(exp_d[:, :], d_raw[:, :], AF.Exp,
                             bias=0.0, scale=1.0 / TAU)
        s_corr = work_p.tile([P, QT], F32, tag="scorr")
        nc.vector.tensor_sub(s_corr[:, :], S_raw[:, :], exp_d[:, :])
        lse = work_p.tile([P, QT], F32, tag="lse")
        nc.scalar.activation(lse[:, :], s_corr[:, :], AF.Ln)
        negpos = work_p.tile([P, QT], F32, tag="negpos")
        nc.vector.tensor_scalar(
            out=negpos[:, :], in0=pos_raw[:, :],
            scalar1=-1.0 / TAU, scalar2=None, op0=ALU.mult,
        )
        loss = work_p.tile([P, QT], F32, tag="loss")
        nc.vector.tensor_add(loss[:, :], lse[:, :], negpos[:, :])
        nc.sync.dma_start(out=out[:, :], in_=loss[:, :])

    _split_excess_waits(nc, mybir)
    return nc


def _get_nc():
    if "nc" not in _CACHE:
        _CACHE["nc"] = _build_nc()
    return _CACHE["nc"]


def kernel(z1, z2):
    from concourse.bass_utils import run_bass_kernel_spmd

    z1 = np.ascontiguousarray(np.asarray(z1, dtype=np.float32))
    z2 = np.ascontiguousarray(np.asarray(z2, dtype=np.float32))
    assert z1.shape == (N, D) and z2.shape == (N, D)

    nc = _get_nc()
    in_maps = [
        {
            "z1": z1,
            "z2": z2,
            "z1q": np.ascontiguousarray(z1[c * NQ:(c + 1) * NQ]),
            "z2q": np.ascontiguousarray(z2[c * NQ:(c + 1) * NQ]),
        }
        for c in range(NCORES)
    ]
    trace = bool(int(os.environ.get("TRNLOSS_TRACE", "0")))
    res = run_bass_kernel_spmd(nc, in_maps, core_ids=list(range(NCORES)), trace=trace)
    if trace:
        _CACHE["exec_time_ns"] = res.exec_time_ns
        print(f"HW exec time: {res.exec_time_ns} ns")
    total = 0.0
    for c in range(NCORES):
        total += res.results[c]["out"].astype(np.float64).sum()
    return np.float32(total / N)



# revision 3
# speedup vs baseline: 1.2390x; 1.2390x over previous
"""NodeContrastiveLoss on 8 Trainium2 NeuronCores (Bass/Tile).

loss = mean_i[ -(z1n_i . z2n_i)/tau
               + log( sum_j exp((z1n_i . z2n_j)/tau)
                    + sum_{j!=i} exp((z1n_i . z1n_j)/tau) ) ]

The Scalar engine's exp stream is the bound (1 elem/lane/cycle @1.2GHz).
This version cuts exp work 22% below the naive split by exploiting the
symmetry of z1n@z1n.T: each unordered block pair {a,b} of the z1-z1
similarity is computed once, in quadrants, so every core's queries are its
OWN 2048 rows and total keys per query row shrink from 32768 to 25600:
  - all 16384 z2 keys           (phase 1, 8 chunks of 2048)
  - the 2048 own-block z1 keys  (phase 1 diag chunk, self term subtracted)
  - 7168 foreign z1 keys        (phase 2: for each peer p, one quadrant
    half per query half, so each z1-z1 off-diag block is computed exactly
    once somewhere)
Row sums come free via the exp accum_out; the transposed contribution of
each off-diag block (column sums) is computed by cheap PE ones-matmuls
(m=32 strips via tile_position) over the bf16 exp block that the Scalar
engine writes to SBUF, accumulated in a dedicated PSUM bank and exported.
The host combines row/col partials across cores and takes log + mean
(device can't: each row's sum needs cross-core parts).

PSUM: phase 1 uses 2x4-bank buffers (2048-key chunks); phase 2 (scoped
pools after release) 2x3-bank buffers (1536-key chunks) + 2x1 colsum bank.
"""

import os
import numpy as np

N, D = 16384, 128
TAU = 0.07
NCORES = 8
NQ = N // NCORES          # 2048 query rows per core
P = 128
QT = NQ // P              # 16 query tiles per core
H = NQ // 2               # 1024: quadrant half
GROUP = 16                # row tiles per staging group (2048 rows)
FD1 = 2048                # phase-1 chunk keys (4 PSUM banks)
FD2 = 1536                # phase-2 max chunk keys (3 PSUM banks)
NCC1 = 9                  # 8 z2 chunks + 1 diag chunk per qtile
CC2 = [1536, 1536, 1536, 1536, 1024]   # phase-2 chunk sizes (sum 7168)
FKEYS = 7168              # foreign keys per quadrant half
NCC = NCC1 + len(CC2)     # 14 accum slots per qtile
SUB = 512                 # matmul moving free dim (one PSUM bank)

_CACHE = {}


def _split_excess_waits(nc, mybir):
    """walrus in this env supports 1 sync-wait per instruction (2 for
    EventSemaphore); move excess waits onto injected same-engine NoOps."""
    n = 0
    for f in nc.m.functions:
        for bb in f.blocks:
            new_insts = None
            for idx, inst in enumerate(bb.instructions):
                si = getattr(inst, "sync_info", None)
                waits = list(si.on_wait) if si is not None and si.on_wait else []
                cap = 2 if getattr(inst, "opcode", None) == "EventSemaphore" else 1
                if len(waits) <= cap:
                    if new_insts is not None:
                        new_insts.append(inst)
                    continue
                if new_insts is None:
                    new_insts = list(bb.instructions[:idx])
                keep, excess = waits[-cap:], waits[:-cap]
                for w in excess:
                    n += 1
                    nop = mybir.InstNoOp(name=f"I-wsplit-{n}-{inst.name}", ins=[], outs=[])
                    nop.engine = inst.engine
                    nop.sync_info = mybir.SyncInfo(on_wait=[w], on_update=[])
                    new_insts.append(nop)
                si.on_wait = keep
                new_insts.append(inst)
            if new_insts is not None:
                bb.instructions = new_insts
    return n


def _build_nc():
    from contextlib import ExitStack

    import concourse.bass as bass
    import concourse.tile as tile
    from concourse import mybir

    F32 = mybir.dt.float32
    BF16 = mybir.dt.bfloat16
    AF = mybir.ActivationFunctionType
    ALU = mybir.AluOpType
    AX = mybir.AxisListType

    nc = bass.Bass("TRN2", target_bir_lowering=False, debug=False)
    z2 = nc.declare_dram_parameter("z2", [N, D], F32, isOutput=False).ap()
    z1q = nc.declare_dram_parameter("z1q", [NQ, D], F32, isOutput=False).ap()
    z2q = nc.declare_dram_parameter("z2q", [NQ, D], F32, isOutput=False).ap()
    fk = nc.declare_dram_parameter("fk", [2 * FKEYS, D], F32, isOutput=False).ap()
    ones32 = nc.declare_dram_parameter("ones32", [P, 32], F32, isOutput=False).ap()
    out = nc.declare_dram_parameter("out", [P, 2 * QT], F32, isOutput=True).ap()
    ucols = nc.declare_dram_parameter("ucols", [2, FKEYS], F32, isOutput=True).ap()

    with tile.TileContext(nc) as tc, ExitStack() as ctx:
        persist = ctx.enter_context(tc.tile_pool(name="persist", bufs=1))
        stage_p = ctx.enter_context(tc.tile_pool(name="stage", bufs=2))
        norm_p = ctx.enter_context(tc.tile_pool(name="norms", bufs=2))
        nbg_p = ctx.enter_context(tc.tile_pool(name="nbg", bufs=2))
        work_p = ctx.enter_context(tc.tile_pool(name="work", bufs=4))
        exp_p = ctx.enter_context(tc.tile_pool(name="expb", bufs=8))
        ucst_p = ctx.enter_context(tc.tile_pool(name="ucst", bufs=2))

        z2T = persist.tile([P, N], BF16, tag="z2T")
        fT = persist.tile([P, 2 * FKEYS], BF16, tag="fT")
        z1qT = persist.tile([P, NQ], BF16, tag="z1qT")
        z1qn = persist.tile([P, NQ], BF16, tag="z1qn")
        z1qnf = persist.tile([P, NQ], F32, tag="z1qnf")
        z2qn = persist.tile([P, NQ], F32, tag="z2qn")
        pos_raw = persist.tile([P, QT], F32, tag="pos")
        d_raw = persist.tile([P, QT], F32, tag="draw")
        part = persist.tile([P, QT * NCC], F32, tag="part")
        onesb = persist.tile([P, 32], BF16, tag="onesb")

        def rsqrt_newton(ssq, ntiles):
            """r = 1/sqrt(ssq) elementwise over [P, ntiles]; ACT ln/exp seed
            (stays in the natural_log_exp table set) + one DVE Newton step."""
            r0 = norm_p.tile([P, GROUP], F32, tag="r0")
            t1 = norm_p.tile([P, GROUP], F32, tag="t1")
            nc.scalar.activation(r0[:, :ntiles], ssq[:, :ntiles], AF.Ln)
            nc.scalar.activation(r0[:, :ntiles], r0[:, :ntiles], AF.Exp,
                                 bias=0.0, scale=-0.5)
            nc.vector.tensor_mul(t1[:, :ntiles], r0[:, :ntiles], r0[:, :ntiles])
            nc.vector.tensor_mul(t1[:, :ntiles], t1[:, :ntiles], ssq[:, :ntiles])
            nc.vector.tensor_scalar(
                out=t1[:, :ntiles], in0=t1[:, :ntiles],
                scalar1=-0.5, scalar2=1.5, op0=ALU.mult, op1=ALU.add,
            )
            nc.vector.tensor_mul(r0[:, :ntiles], r0[:, :ntiles], t1[:, :ntiles])
            return r0

        def load_group(src, row0, ntiles):
            """DMA ntiles row tiles to staging; compute 1/norm per row."""
            stage = stage_p.tile([P, GROUP, P], F32, tag="stage")
            nc.sync.dma_start(
                out=stage[:, :ntiles, :],
                in_=src[row0:row0 + ntiles * P, :].rearrange("(t p) d -> p t d", p=P),
            )
            ssq = norm_p.tile([P, GROUP], F32, tag="ssq")
            for t in range(ntiles):
                sq = work_p.tile([P, P], F32, tag="sq")
                nc.vector.scalar_tensor_tensor(
                    out=sq[:, :], in0=stage[:, t, :], scalar=1.0,
                    in1=stage[:, t, :], op0=ALU.bypass, op1=ALU.mult,
                    accum_out=ssq[:, t:t + 1],
                )
            return stage, rsqrt_newton(ssq, ntiles)

        def normalize_group(stage, r, ntiles):
            nbg = nbg_p.tile([P, GROUP * P], BF16, tag="nbg")
            for t in range(ntiles):
                nc.vector.tensor_scalar_mul(
                    nbg[:, t * P:(t + 1) * P], stage[:, t, :], r[:, t:t + 1])
            return nbg

        def transpose_group(nbg, dst_T, col0, ntiles):
            dst3 = dst_T[:, col0:col0 + ntiles * P].rearrange(
                "p (t d) -> p t d", d=P)
            nc.sync.dma_start_transpose(dst3, nbg[:, :ntiles * P])

        def stage_keys(src, row0, dst_T, col0):
            stg, r = load_group(src, row0, GROUP)
            nbg = normalize_group(stg, r, GROUP)
            transpose_group(nbg, dst_T, col0, GROUP)

        def deferred_qprep():
            """z2q chain + pos + d: runs in engine slack under early exps."""
            stg, rq = load_group(z2q, 0, QT)
            for t in range(QT):
                nc.vector.tensor_scalar_mul(
                    z2qn[:, t * P:(t + 1) * P], stg[:, t, :], rq[:, t:t + 1])
            for t in range(QT):
                # d_raw[:, t] = sum_d bf16(z1n)^2 (matches the PE diag dot)
                sq = work_p.tile([P, P], F32, tag="dsq")
                nc.gpsimd.tensor_mul(sq[:, :], z1qn[:, t * P:(t + 1) * P],
                                     z1qn[:, t * P:(t + 1) * P])
                nc.vector.tensor_reduce(
                    out=d_raw[:, t:t + 1], in_=sq[:, :], axis=AX.X, op=ALU.add)
                # pos_raw[:, t] = sum_d z1n * z2n (f32)
                mb = work_p.tile([P, P], F32, tag="mb")
                nc.gpsimd.tensor_mul(mb[:, :], z1qnf[:, t * P:(t + 1) * P],
                                     z2qn[:, t * P:(t + 1) * P])
                nc.vector.tensor_reduce(
                    out=pos_raw[:, t:t + 1], in_=mb[:, :], axis=AX.X, op=ALU.add)

        # ---------------- prologue ----------------
        # own z1 block: bf16 transposed (lhsT for all matmuls + diag keys),
        # bf16 row-major (for d), f32 row-major (for pos)
        stage, r = load_group(z1q, 0, QT)
        for t in range(QT):
            nc.vector.tensor_scalar_mul(
                z1qn[:, t * P:(t + 1) * P], stage[:, t, :], r[:, t:t + 1])
        transpose_group(z1qn, z1qT, 0, QT)
        for t in range(QT):
            nc.vector.tensor_scalar_mul(
                z1qnf[:, t * P:(t + 1) * P], stage[:, t, :], r[:, t:t + 1])
        # first z2 group, so phase-1 cc0 can start immediately
        stage_keys(z2, 0, z2T, 0)
        # ones (bf16) for the colsum matmuls
        onesf = persist.tile([P, 32], F32, tag="onesf")
        nc.sync.dma_start(out=onesf[:, :], in_=ones32[:, :])
        nc.vector.tensor_copy(onesb[:, :], onesf[:, :])

        # ---------------- phase 1: z2 keys + diag chunk (FD1=2048) --------
        # Staging chains for later chunk-columns are emitted a few chunks
        # into each column: their rsqrt Ln/Exp land in the ACT FIFO behind
        # already-runnable exp chunks, so the in-order ACT queue never
        # blocks on a staging dependency (the baseline lost ~40us to this).
        with tc.tile_pool(name="ps1", bufs=2, space="PSUM") as ps1:
            for cc in range(NCC1):
                keysT, koff = (z2T, cc * FD1) if cc < 8 else (z1qT, 0)
                for qt in range(QT):
                    if qt == 3 and cc <= 6:
                        stage_keys(z2, (cc + 1) * GROUP * P, z2T,
                                   (cc + 1) * GROUP * P)
                    if qt == 8 and cc == 0:
                        deferred_qprep()
                    if qt == 8 and 2 <= cc <= 8:
                        g = cc - 2
                        stage_keys(fk, g * GROUP * P, fT, g * GROUP * P)
                    ps = ps1.tile([P, FD1], F32, tag="ps")
                    for j in range(FD1 // SUB):
                        nc.tensor.matmul(
                            ps[:, j * SUB:(j + 1) * SUB],
                            lhsT=z1qT[:, qt * P:(qt + 1) * P],
                            rhs=keysT[:, koff + j * SUB:koff + (j + 1) * SUB],
                            start=True, stop=True,
                        )
                    nc.scalar.activation(
                        ps[:, :], ps[:, :], AF.Exp, bias=0.0, scale=1.0 / TAU,
                        accum_out=part[:, qt * NCC + cc:qt * NCC + cc + 1],
                    )

        # ---------------- phase 2: foreign keys (symmetric off-diag) ------
        # chunk (c2, qh, k): queries = own qtile qh*8+k, keys = fT slice.
        # exp -> SBUF bf16 (rhs for colsum matmuls) + accum_out row sums.
        # colsum matmuls for batch (c2, qh) are emitted interleaved into the
        # NEXT batch's slots (deps already satisfied -> no PE stall), with
        # m=32 ones-lhsT strips accumulating into a 1-bank PSUM tile.
        def emit_colsum(prev, k):
            pcs, pebs, pfd = prev
            for s in range(pfd // SUB):
                nc.tensor.matmul(
                    pcs[32 * s:32 * s + 32, :],
                    lhsT=onesb[:, 0:32],
                    rhs=pebs[k][:, s * SUB:(s + 1) * SUB],
                    start=(k == 0), stop=(k == QT // 2 - 1),
                    tile_position=(0, 32 * s),
                )

        def emit_drain(prev, pqh, pc2):
            pcs, _, pfd = prev
            nst = pfd // SUB
            uc = ucst_p.tile([P, SUB], F32, tag="ucst")
            nc.vector.tensor_copy(uc[:, :], pcs[:, :])
            base = pc2 * FD2
            for s in range(nst):
                nc.sync.dma_start(
                    out=ucols[pqh:pqh + 1, base + s * SUB:base + (s + 1) * SUB],
                    in_=uc[32 * s:32 * s + 1, :],
                )

        with tc.tile_pool(name="ps2", bufs=2, space="PSUM") as ps2, \
                tc.tile_pool(name="cs", bufs=2, space="PSUM") as cs_p:
            prev = None      # (cs_tile, exp_tiles, fd) of previous batch
            prev_loc = None  # (qh, c2)
            for c2 in range(len(CC2)):
                fd = CC2[c2]
                for qh in range(2):
                    cs = cs_p.tile([P, SUB], F32, tag="cs")
                    ebs = []
                    for k in range(QT // 2):
                        qt = qh * (QT // 2) + k
                        koff = qh * FKEYS + c2 * FD2
                        ps = ps2.tile([P, FD2], F32, tag="ps")
                        for j in range(fd // SUB):
                            nc.tensor.matmul(
                                ps[:, j * SUB:(j + 1) * SUB],
                                lhsT=z1qT[:, qt * P:(qt + 1) * P],
                                rhs=fT[:, koff + j * SUB:koff + (j + 1) * SUB],
                                start=True, stop=True,
                            )
                        eb = exp_p.tile([P, FD2], BF16, tag="eb")
                        # previous batch's colsum matmuls: WAR-ordered before
                        # this ACT overwrites the rotated exp buffer
                        if prev is not None:
                            emit_colsum(prev, k)
                        nc.scalar.activation(
                            eb[:, :fd], ps[:, :fd], AF.Exp,
                            bias=0.0, scale=1.0 / TAU,
                            accum_out=part[:, qt * NCC + NCC1 + c2:
                                           qt * NCC + NCC1 + c2 + 1],
                        )
                        ebs.append(eb)
                    if prev is not None:
                        emit_drain(prev, *prev_loc)
                    prev, prev_loc = (cs, ebs, fd), (qh, c2)
            # last batch's colsums + drain (PE/DMA tail under the epilogue)
            for k in range(QT // 2):
                emit_colsum(prev, k)
            emit_drain(prev, *prev_loc)

        # ---------------- epilogue: per-row partial sums ----------------
        S_own = work_p.tile([P, QT], F32, tag="sown")
        for qt in range(QT):
            nc.vector.tensor_reduce(
                out=S_own[:, qt:qt + 1],
                in_=part[:, qt * NCC:(qt + 1) * NCC],
                axis=AX.X, op=ALU.add,
            )
        exp_d = work_p.tile([P, QT], F32, tag="expd")
        nc.scalar.activation(exp_d[:, :], d_raw[:, :], AF.Exp,
                             bias=0.0, scale=1.0 / TAU)
        res = work_p.tile([P, 2 * QT], F32, tag="res")
        nc.vector.tensor_sub(res[:, 0:QT], S_own[:, :], exp_d[:, :])
        nc.vector.tensor_copy(res[:, QT:2 * QT], pos_raw[:, :])
        nc.sync.dma_start(out=out[:, :], in_=res[:, :])

    _split_excess_waits(nc, mybir)
    return nc


def _get_nc():
    if "nc" not in _CACHE:
        _CACHE["nc"] = _build_nc()
    return _CACHE["nc"]


def _foreign_rows(c):
    """Per-core foreign key row indices: [qh, u] -> H rows of z1."""
    rows = []
    for qh in range(2):
        for d in range(1, NCORES):
            p = (c + d) % NCORES
            half = qh if c < p else 1 - qh
            off = p * NQ + half * H
            rows.append(np.arange(off, off + H))
    return np.concatenate(rows)


def kernel(z1, z2):
    from concourse.bass_utils import run_bass_kernel_spmd

    z1 = np.ascontiguousarray(np.asarray(z1, dtype=np.float32))
    z2 = np.ascontiguousarray(np.asarray(z2, dtype=np.float32))
    assert z1.shape == (N, D) and z2.shape == (N, D)

    nc = _get_nc()
    ones = np.ones((P, 32), dtype=np.float32)
    in_maps = [
        {
            "z2": z2,
            "z1q": np.ascontiguousarray(z1[c * NQ:(c + 1) * NQ]),
            "z2q": np.ascontiguousarray(z2[c * NQ:(c + 1) * NQ]),
            "fk": np.ascontiguousarray(z1[_foreign_rows(c)]),
            "ones32": ones,
        }
        for c in range(NCORES)
    ]
    trace = bool(int(os.environ.get("TRNLOSS_TRACE", "0")))
    res = run_bass_kernel_spmd(nc, in_maps, core_ids=list(range(NCORES)), trace=trace)
    if trace:
        _CACHE["exec_time_ns"] = res.exec_time_ns
        print(f"HW exec time: {res.exec_time_ns} ns")

    S = np.zeros(N, dtype=np.float64)
    pos = np.zeros(N, dtype=np.float64)
    for c in range(NCORES):
        o = res.results[c]["out"].astype(np.float64)      # [P, 2*QT]
        S[c * NQ:(c + 1) * NQ] += o[:, 0:QT].T.reshape(-1)
        pos[c * NQ:(c + 1) * NQ] = o[:, QT:2 * QT].T.reshape(-1)
        uc = res.results[c]["ucols"].astype(np.float64)   # [2, FKEYS]
        for qh in range(2):
            for u in range(NCORES - 1):
                d = u + 1
                p = (c + d) % NCORES
                half = qh if c < p else 1 - qh
                off = p * NQ + half * H
                S[off:off + H] += uc[qh, u * H:(u + 1) * H]
    loss = np.mean(np.log(S) - pos / TAU)
    return np.float32(loss)


# revision 7
# speedup vs baseline: 1.2503x; 1.0091x over previous
"""NodeContrastiveLoss on 8 Trainium2 NeuronCores (Bass/Tile).

loss = mean_i[ -(z1n_i . z2n_i)/tau
               + log( sum_j exp((z1n_i . z2n_j)/tau)
                    + sum_{j!=i} exp((z1n_i . z1n_j)/tau) ) ]

The Scalar engine's exp stream is the bound (1 elem/lane/cycle @1.2GHz).
This version cuts exp work 22% below the naive split by exploiting the
symmetry of z1n@z1n.T: each unordered block pair {a,b} of the z1-z1
similarity is computed once, in quadrants, so every core's queries are its
OWN 2048 rows and total keys per query row shrink from 32768 to 25600:
  - all 16384 z2 keys           (phase 1, 8 chunks of 2048)
  - the 2048 own-block z1 keys  (phase 1 diag chunk, self term subtracted)
  - 7168 foreign z1 keys        (phase 2: for each peer p, one quadrant
    half per query half, so each z1-z1 off-diag block is computed exactly
    once somewhere)
Row sums come free via the exp accum_out; the transposed contribution of
each off-diag block (column sums) is computed by cheap PE ones-matmuls
(m=32 strips via tile_position) over the bf16 exp block that the Scalar
engine writes to SBUF, accumulated in a dedicated PSUM bank and exported.
The host combines row/col partials across cores and takes log + mean
(device can't: each row's sum needs cross-core parts).

PSUM: phase 1 uses 2x4-bank buffers (2048-key chunks); phase 2 (scoped
pools after release) 2x3-bank buffers (1536-key chunks) + 2x1 colsum bank.
"""

import os
import numpy as np

N, D = 16384, 128
TAU = 0.07
NCORES = 8
NQ = N // NCORES          # 2048 query rows per core
P = 128
QT = NQ // P              # 16 query tiles per core
H = NQ // 2               # 1024: quadrant half
GROUP = 16                # row tiles per staging group (2048 rows)
FD1 = 2048                # phase-1 chunk keys (4 PSUM banks)
FD2 = 1536                # phase-2 max chunk keys (3 PSUM banks)
NCC1 = 9                  # 8 z2 chunks + 1 diag chunk per qtile
CC2 = [1536, 1536, 1536, 1536, 1024]   # phase-2 chunk sizes (sum 7168)
FKEYS = 7168              # foreign keys per quadrant half
NCC = NCC1 + len(CC2)     # 14 accum slots per qtile
SUB = 512                 # matmul moving free dim (one PSUM bank)

_CACHE = {}


def _split_excess_waits(nc, mybir):
    """walrus in this env supports 1 sync-wait per instruction (2 for
    EventSemaphore); move excess waits onto injected same-engine NoOps."""
    n = 0
    for f in nc.m.functions:
        for bb in f.blocks:
            new_insts = None
            for idx, inst in enumerate(bb.instructions):
                si = getattr(inst, "sync_info", None)
                waits = list(si.on_wait) if si is not None and si.on_wait else []
                cap = 2 if getattr(inst, "opcode", None) == "EventSemaphore" else 1
                if len(waits) <= cap:
                    if new_insts is not None:
                        new_insts.append(inst)
                    continue
                if new_insts is None:
                    new_insts = list(bb.instructions[:idx])
                keep, excess = waits[-cap:], waits[:-cap]
                for w in excess:
                    n += 1
                    nop = mybir.InstNoOp(name=f"I-wsplit-{n}-{inst.name}", ins=[], outs=[])
                    nop.engine = inst.engine
                    nop.sync_info = mybir.SyncInfo(on_wait=[w], on_update=[])
                    new_insts.append(nop)
                si.on_wait = keep
                new_insts.append(inst)
            if new_insts is not None:
                bb.instructions = new_insts
    return n


def _build_nc():
    from contextlib import ExitStack

    import concourse.bass as bass
    import concourse.tile as tile
    from concourse import mybir

    F32 = mybir.dt.float32
    BF16 = mybir.dt.bfloat16
    AF = mybir.ActivationFunctionType
    ALU = mybir.AluOpType
    AX = mybir.AxisListType

    nc = bass.Bass("TRN2", target_bir_lowering=False, debug=False)
    z2 = nc.declare_dram_parameter("z2", [N, D], F32, isOutput=False).ap()
    z1q = nc.declare_dram_parameter("z1q", [NQ, D], F32, isOutput=False).ap()
    z2q = nc.declare_dram_parameter("z2q", [NQ, D], F32, isOutput=False).ap()
    fk = nc.declare_dram_parameter("fk", [2 * FKEYS, D], F32, isOutput=False).ap()
    ones32 = nc.declare_dram_parameter("ones32", [P, 32], F32, isOutput=False).ap()
    out = nc.declare_dram_parameter("out", [P, 2 * QT], F32, isOutput=True).ap()
    ucols = nc.declare_dram_parameter("ucols", [2, FKEYS], F32, isOutput=True).ap()

    with tile.TileContext(nc) as tc, ExitStack() as ctx:
        persist = ctx.enter_context(tc.tile_pool(name="persist", bufs=1))
        stage_p = ctx.enter_context(tc.tile_pool(name="stage", bufs=2))
        norm_p = ctx.enter_context(tc.tile_pool(name="norms", bufs=2))
        nbg_p = ctx.enter_context(tc.tile_pool(name="nbg", bufs=2))
        work_p = ctx.enter_context(tc.tile_pool(name="work", bufs=4))
        exp_p = ctx.enter_context(tc.tile_pool(name="expb", bufs=8))
        ucst_p = ctx.enter_context(tc.tile_pool(name="ucst", bufs=2))

        z2T = persist.tile([P, N], BF16, tag="z2T")
        fT = persist.tile([P, 2 * FKEYS], BF16, tag="fT")
        z1qT = persist.tile([P, NQ], BF16, tag="z1qT")
        z1qn = persist.tile([P, NQ], BF16, tag="z1qn")
        z1qnf = persist.tile([P, NQ], F32, tag="z1qnf")
        z2qn = persist.tile([P, NQ], F32, tag="z2qn")
        pos_raw = persist.tile([P, QT], F32, tag="pos")
        d_raw = persist.tile([P, QT], F32, tag="draw")
        part = persist.tile([P, QT * NCC], F32, tag="part")
        onesb = persist.tile([P, 32], BF16, tag="onesb")

        def rsqrt_newton(ssq, ntiles):
            """r = 1/sqrt(ssq) elementwise over [P, ntiles]; ACT ln/exp seed
            (stays in the natural_log_exp table set) + one DVE Newton step."""
            r0 = norm_p.tile([P, GROUP], F32, tag="r0")
            t1 = norm_p.tile([P, GROUP], F32, tag="t1")
            nc.scalar.activation(r0[:, :ntiles], ssq[:, :ntiles], AF.Ln)
            nc.scalar.activation(r0[:, :ntiles], r0[:, :ntiles], AF.Exp,
                                 bias=0.0, scale=-0.5)
            nc.vector.tensor_mul(t1[:, :ntiles], r0[:, :ntiles], r0[:, :ntiles])
            nc.vector.tensor_mul(t1[:, :ntiles], t1[:, :ntiles], ssq[:, :ntiles])
            nc.vector.tensor_scalar(
                out=t1[:, :ntiles], in0=t1[:, :ntiles],
                scalar1=-0.5, scalar2=1.5, op0=ALU.mult, op1=ALU.add,
            )
            nc.vector.tensor_mul(r0[:, :ntiles], r0[:, :ntiles], t1[:, :ntiles])
            return r0

        def load_raw(src, row0, ntiles):
            """DMA ntiles row tiles to staging; per-row sum of squares."""
            stage = stage_p.tile([P, GROUP, P], F32, tag="stage")
            nc.sync.dma_start(
                out=stage[:, :ntiles, :],
                in_=src[row0:row0 + ntiles * P, :].rearrange("(t p) d -> p t d", p=P),
            )
            ssq = norm_p.tile([P, GROUP], F32, tag="ssq")
            for t in range(ntiles):
                sq = work_p.tile([P, P], F32, tag="sq")
                nc.vector.scalar_tensor_tensor(
                    out=sq[:, :], in0=stage[:, t, :], scalar=1.0,
                    in1=stage[:, t, :], op0=ALU.bypass, op1=ALU.mult,
                    accum_out=ssq[:, t:t + 1],
                )
            return stage, ssq

        def load_group(src, row0, ntiles):
            stage, ssq = load_raw(src, row0, ntiles)
            return stage, rsqrt_newton(ssq, ntiles)

        def normalize_group(stage, r, ntiles):
            nbg = nbg_p.tile([P, GROUP * P], BF16, tag="nbg")
            for t in range(ntiles):
                nc.vector.tensor_scalar_mul(
                    nbg[:, t * P:(t + 1) * P], stage[:, t, :], r[:, t:t + 1])
            return nbg

        def transpose_group(nbg, dst_T, col0, ntiles):
            dst3 = dst_T[:, col0:col0 + ntiles * P].rearrange(
                "p (t d) -> p t d", d=P)
            nc.sync.dma_start_transpose(dst3, nbg[:, :ntiles * P])

        def finish_keys(stg, ssq, dst_T, col0):
            r = rsqrt_newton(ssq, GROUP)
            nbg = normalize_group(stg, r, GROUP)
            transpose_group(nbg, dst_T, col0, GROUP)

        def stage_keys(src, row0, dst_T, col0):
            stg, ssq = load_raw(src, row0, GROUP)
            finish_keys(stg, ssq, dst_T, col0)

        def deferred_qprep():
            """z2q chain + pos + d: runs in engine slack under early exps."""
            stg, rq = load_group(z2q, 0, QT)
            for t in range(QT):
                nc.vector.tensor_scalar_mul(
                    z2qn[:, t * P:(t + 1) * P], stg[:, t, :], rq[:, t:t + 1])
            for t in range(QT):
                # d_raw[:, t] = sum_d bf16(z1n)^2 (matches the PE diag dot)
                sq = work_p.tile([P, P], F32, tag="dsq")
                nc.gpsimd.tensor_mul(sq[:, :], z1qn[:, t * P:(t + 1) * P],
                                     z1qn[:, t * P:(t + 1) * P])
                nc.vector.tensor_reduce(
                    out=d_raw[:, t:t + 1], in_=sq[:, :], axis=AX.X, op=ALU.add)
                # pos_raw[:, t] = sum_d z1n * z2n (f32)
                mb = work_p.tile([P, P], F32, tag="mb")
                nc.gpsimd.tensor_mul(mb[:, :], z1qnf[:, t * P:(t + 1) * P],
                                     z2qn[:, t * P:(t + 1) * P])
                nc.vector.tensor_reduce(
                    out=pos_raw[:, t:t + 1], in_=mb[:, :], axis=AX.X, op=ALU.add)

        # ---------------- prologue ----------------
        # Critical path to the first exp chunk is ONLY the own-block chain:
        # phase-1 cc0 is the diag chunk, whose keys are z1qT itself.  The
        # z2 g0 DMA+squares start concurrently, but g0's rsqrt Ln is NOT
        # emitted here — it would sit in the in-order ACT FIFO ahead of
        # cc0's already-runnable exps.  It is emitted under cc0 (qt==2).
        stage, ssqQ = load_raw(z1q, 0, QT)
        st0, sq0 = load_raw(z2, 0, GROUP)
        r = rsqrt_newton(ssqQ, QT)
        for t in range(QT):
            nc.vector.tensor_scalar_mul(
                z1qn[:, t * P:(t + 1) * P], stage[:, t, :], r[:, t:t + 1])
        transpose_group(z1qn, z1qT, 0, QT)
        for t in range(QT):
            nc.vector.tensor_scalar_mul(
                z1qnf[:, t * P:(t + 1) * P], stage[:, t, :], r[:, t:t + 1])
        # ones (bf16) for the colsum matmuls
        onesf = persist.tile([P, 32], F32, tag="onesf")
        nc.sync.dma_start(out=onesf[:, :], in_=ones32[:, :])
        nc.vector.tensor_copy(onesb[:, :], onesf[:, :])

        # ---------------- phase 1: diag chunk + z2 keys (FD1=2048) --------
        # cc0 = diag (keys z1qT), cc 1..8 = z2 groups 0..7.  Staging chains
        # for later chunk-columns are emitted a few chunks into each column:
        # their rsqrt Ln/Exp land in the ACT FIFO behind already-runnable
        # exp chunks, so the in-order ACT queue never blocks on a staging
        # dependency (the baseline lost ~40us to this).
        with tc.tile_pool(name="ps1", bufs=2, space="PSUM") as ps1:
            for cc in range(NCC1):
                keysT, koff = (z1qT, 0) if cc == 0 else (z2T, (cc - 1) * FD1)
                for qt in range(QT):
                    if qt == 2 and cc == 0:
                        finish_keys(st0, sq0, z2T, 0)
                    if qt == 6 and cc <= 6:
                        stage_keys(z2, (cc + 1) * GROUP * P, z2T,
                                   (cc + 1) * GROUP * P)
                    if qt == 10 and cc == 0:
                        deferred_qprep()
                    if qt == 12 and 2 <= cc <= 8:
                        g = cc - 2
                        stage_keys(fk, g * GROUP * P, fT, g * GROUP * P)
                    ps = ps1.tile([P, FD1], F32, tag="ps")
                    for j in range(FD1 // SUB):
                        nc.tensor.matmul(
                            ps[:, j * SUB:(j + 1) * SUB],
                            lhsT=z1qT[:, qt * P:(qt + 1) * P],
                            rhs=keysT[:, koff + j * SUB:koff + (j + 1) * SUB],
                            start=True, stop=True,
                        )
                    nc.scalar.activation(
                        ps[:, :], ps[:, :], AF.Exp, bias=0.0, scale=1.0 / TAU,
                        accum_out=part[:, qt * NCC + cc:qt * NCC + cc + 1],
                    )

        # ---------------- phase 2: foreign keys (symmetric off-diag) ------
        # chunk (c2, qh, k): queries = own qtile qh*8+k, keys = fT slice.
        # exp -> SBUF bf16 (rhs for colsum matmuls) + accum_out row sums.
        # colsum matmuls for batch (c2, qh) are emitted interleaved into the
        # NEXT batch's slots (deps already satisfied -> no PE stall), with
        # m=32 ones-lhsT strips accumulating into a 1-bank PSUM tile.
        def emit_colsum(prev, k):
            pcs, pebs, pfd = prev
            for s in range(pfd // SUB):
                nc.tensor.matmul(
                    pcs[32 * s:32 * s + 32, :],
                    lhsT=onesb[:, 0:32],
                    rhs=pebs[k][:, s * SUB:(s + 1) * SUB],
                    start=(k == 0), stop=(k == QT // 2 - 1),
                    tile_position=(0, 32 * s),
                )

        def emit_drain(prev, pqh, pc2):
            pcs, _, pfd = prev
            nst = pfd // SUB
            uc = ucst_p.tile([P, SUB], F32, tag="ucst")
            nc.vector.tensor_copy(uc[:, :], pcs[:, :])
            base = pc2 * FD2
            for s in range(nst):
                nc.sync.dma_start(
                    out=ucols[pqh:pqh + 1, base + s * SUB:base + (s + 1) * SUB],
                    in_=uc[32 * s:32 * s + 1, :],
                )

        with tc.tile_pool(name="ps2", bufs=2, space="PSUM") as ps2, \
                tc.tile_pool(name="cs", bufs=2, space="PSUM") as cs_p:
            prev = None      # (cs_tile, exp_tiles, fd) of previous batch
            prev_loc = None  # (qh, c2)
            for c2 in range(len(CC2)):
                fd = CC2[c2]
                for qh in range(2):
                    cs = cs_p.tile([P, SUB], F32, tag="cs")
                    ebs = []
                    for k in range(QT // 2):
                        qt = qh * (QT // 2) + k
                        koff = qh * FKEYS + c2 * FD2
                        ps = ps2.tile([P, FD2], F32, tag="ps")
                        for j in range(fd // SUB):
                            nc.tensor.matmul(
                                ps[:, j * SUB:(j + 1) * SUB],
                                lhsT=z1qT[:, qt * P:(qt + 1) * P],
                                rhs=fT[:, koff + j * SUB:koff + (j + 1) * SUB],
                                start=True, stop=True,
                            )
                        eb = exp_p.tile([P, FD2], BF16, tag="eb")
                        # previous batch's colsum matmuls: WAR-ordered before
                        # this ACT overwrites the rotated exp buffer
                        if prev is not None:
                            emit_colsum(prev, k)
                        nc.scalar.activation(
                            eb[:, :fd], ps[:, :fd], AF.Exp,
                            bias=0.0, scale=1.0 / TAU,
                            accum_out=part[:, qt * NCC + NCC1 + c2:
                                           qt * NCC + NCC1 + c2 + 1],
                        )
                        ebs.append(eb)
                    if prev is not None:
                        emit_drain(prev, *prev_loc)
                    prev, prev_loc = (cs, ebs, fd), (qh, c2)
            # last batch's colsums + drain (PE/DMA tail under the epilogue)
            for k in range(QT // 2):
                emit_colsum(prev, k)
            emit_drain(prev, *prev_loc)

        # ---------------- epilogue: per-row partial sums ----------------
        S_own = work_p.tile([P, QT], F32, tag="sown")
        for qt in range(QT):
            nc.vector.tensor_reduce(
                out=S_own[:, qt:qt + 1],
                in_=part[:, qt * NCC:(qt + 1) * NCC],
                axis=AX.X, op=ALU.add,
            )
        exp_d = work_p.tile([P, QT], F32, tag="expd")
        nc.scalar.activation(exp_d[:, :], d_raw[:, :], AF.Exp,
                             bias=0.0, scale=1.0 / TAU)
        res = work_p.tile([P, 2 * QT], F32, tag="res")
        nc.vector.tensor_sub(res[:, 0:QT], S_own[:, :], exp_d[:, :])
        nc.vector.tensor_copy(res[:, QT:2 * QT], pos_raw[:, :])
        nc.sync.dma_start(out=out[:, :], in_=res[:, :])

    _split_excess_waits(nc, mybir)
    return nc


def _get_nc():
    if "nc" not in _CACHE:
        _CACHE["nc"] = _build_nc()
    return _CACHE["nc"]


def _foreign_rows(c):
    """Per-core foreign key row indices: [qh, u] -> H rows of z1."""
    rows = []
    for qh in range(2):
        for d in range(1, NCORES):
            p = (c + d) % NCORES
            half = qh if c < p else 1 - qh
            off = p * NQ + half * H
            rows.append(np.arange(off, off + H))
    return np.concatenate(rows)


def kernel(z1, z2):
    from concourse.bass_utils import run_bass_kernel_spmd

    z1 = np.ascontiguousarray(np.asarray(z1, dtype=np.float32))
    z2 = np.ascontiguousarray(np.asarray(z2, dtype=np.float32))
    assert z1.shape == (N, D) and z2.shape == (N, D)

    nc = _get_nc()
    ones = np.ones((P, 32), dtype=np.float32)
    in_maps = [
        {
            "z2": z2,
            "z1q": np.ascontiguousarray(z1[c * NQ:(c + 1) * NQ]),
            "z2q": np.ascontiguousarray(z2[c * NQ:(c + 1) * NQ]),
            "fk": np.ascontiguousarray(z1[_foreign_rows(c)]),
            "ones32": ones,
        }
        for c in range(NCORES)
    ]
    trace = bool(int(os.environ.get("TRNLOSS_TRACE", "0")))
    res = run_bass_kernel_spmd(nc, in_maps, core_ids=list(range(NCORES)), trace=trace)
    if trace:
        _CACHE["exec_time_ns"] = res.exec_time_ns
        print(f"HW exec time: {res.exec_time_ns} ns")

    S = np.zeros(N, dtype=np.float64)
    pos = np.zeros(N, dtype=np.float64)
    for c in range(NCORES):
        o = res.results[c]["out"].astype(np.float64)      # [P, 2*QT]
        S[c * NQ:(c + 1) * NQ] += o[:, 0:QT].T.reshape(-1)
        pos[c * NQ:(c + 1) * NQ] = o[:, QT:2 * QT].T.reshape(-1)
        uc = res.results[c]["ucols"].astype(np.float64)   # [2, FKEYS]
        for qh in range(2):
            for u in range(NCORES - 1):
                d = u + 1
                p = (c + d) % NCORES
                half = qh if c < p else 1 - qh
                off = p * NQ + half * H
                S[off:off + H] += uc[qh, u * H:(u + 1) * H]
    loss = np.mean(np.log(S) - pos / TAU)
    return np.float32(loss)


# revision 10
# speedup vs baseline: 1.2632x; 1.0104x over previous
"""NodeContrastiveLoss on 8 Trainium2 NeuronCores (Bass/Tile).

loss = mean_i[ -(z1n_i . z2n_i)/tau
               + log( sum_j exp((z1n_i . z2n_j)/tau)
                    + sum_{j!=i} exp((z1n_i . z1n_j)/tau) ) ]

The Scalar engine's exp stream is the bound (1 elem/lane/cycle @1.2GHz).
This version cuts exp work 22% below the naive split by exploiting the
symmetry of z1n@z1n.T: each unordered block pair {a,b} of the z1-z1
similarity is computed once, in quadrants, so every core's queries are its
OWN 2048 rows and total keys per query row shrink from 32768 to 25600:
  - all 16384 z2 keys           (phase 1, 8 chunks of 2048)
  - the 2048 own-block z1 keys  (phase 1 diag chunk, self term subtracted)
  - 7168 foreign z1 keys        (phase 2: for each peer p, one quadrant
    half per query half, so each z1-z1 off-diag block is computed exactly
    once somewhere)
Row sums come free via the exp accum_out; the transposed contribution of
each off-diag block (column sums) is computed by cheap PE ones-matmuls
(m=32 strips via tile_position) over the bf16 exp block that the Scalar
engine writes to SBUF, accumulated in a dedicated PSUM bank and exported.
The host combines row/col partials across cores and takes log + mean
(device can't: each row's sum needs cross-core parts).

PSUM: phase 1 uses 2x4-bank buffers (2048-key chunks); phase 2 (scoped
pools after release) 2x3-bank buffers (1536-key chunks) + 2x1 colsum bank.
"""

import os
import numpy as np

N, D = 16384, 128
TAU = 0.07
NCORES = 8
NQ = N // NCORES          # 2048 query rows per core
P = 128
QT = NQ // P              # 16 query tiles per core
H = NQ // 2               # 1024: quadrant half
GROUP = 16                # row tiles per staging group (2048 rows)
FD1 = 2048                # phase-1 chunk keys (4 PSUM banks)
FD2 = 1536                # phase-2 max chunk keys (3 PSUM banks)
NCC1 = 9                  # 8 z2 chunks + 1 diag chunk per qtile
CC2 = [1536, 1536, 1536, 1536, 1024]   # phase-2 chunk sizes (sum 7168)
FKEYS = 7168              # foreign keys per quadrant half
NCC = NCC1 + len(CC2)     # 14 accum slots per qtile
SUB = 512                 # matmul moving free dim (one PSUM bank)

_CACHE = {}


def _split_excess_waits(nc, mybir):
    """walrus in this env supports 1 sync-wait per instruction (2 for
    EventSemaphore); move excess waits onto injected same-engine NoOps."""
    n = 0
    for f in nc.m.functions:
        for bb in f.blocks:
            new_insts = None
            for idx, inst in enumerate(bb.instructions):
                si = getattr(inst, "sync_info", None)
                waits = list(si.on_wait) if si is not None and si.on_wait else []
                cap = 2 if getattr(inst, "opcode", None) == "EventSemaphore" else 1
                if len(waits) <= cap:
                    if new_insts is not None:
                        new_insts.append(inst)
                    continue
                if new_insts is None:
                    new_insts = list(bb.instructions[:idx])
                keep, excess = waits[-cap:], waits[:-cap]
                for w in excess:
                    n += 1
                    nop = mybir.InstNoOp(name=f"I-wsplit-{n}-{inst.name}", ins=[], outs=[])
                    nop.engine = inst.engine
                    nop.sync_info = mybir.SyncInfo(on_wait=[w], on_update=[])
                    new_insts.append(nop)
                si.on_wait = keep
                new_insts.append(inst)
            if new_insts is not None:
                bb.instructions = new_insts
    return n


def _build_nc():
    from contextlib import ExitStack

    import concourse.bass as bass
    import concourse.tile as tile
    from concourse import mybir

    F32 = mybir.dt.float32
    BF16 = mybir.dt.bfloat16
    AF = mybir.ActivationFunctionType
    ALU = mybir.AluOpType
    AX = mybir.AxisListType

    nc = bass.Bass("TRN2", target_bir_lowering=False, debug=False)
    # All row-tensors arrive HOST-PRE-TRANSPOSED to [128, rows] layout
    # (arr[p, t*128+d] = rows[t*128+p, d]) so every load is one contiguous
    # 8KB-per-partition DMA descriptor instead of 512B-row gathers (the
    # strided version made the first load ~13us and kept DMA rings busy
    # with descriptor overhead all run).
    z2 = nc.declare_dram_parameter("z2", [P, N], F32, isOutput=False).ap()
    z1q = nc.declare_dram_parameter("z1q", [P, NQ], F32, isOutput=False).ap()
    z2q = nc.declare_dram_parameter("z2q", [P, NQ], F32, isOutput=False).ap()
    fk = nc.declare_dram_parameter("fk", [P, 2 * FKEYS], F32, isOutput=False).ap()
    ones32 = nc.declare_dram_parameter("ones32", [P, 32], F32, isOutput=False).ap()
    out = nc.declare_dram_parameter("out", [P, 2 * QT], F32, isOutput=True).ap()
    ucols = nc.declare_dram_parameter("ucols", [2, FKEYS], F32, isOutput=True).ap()

    with tile.TileContext(nc) as tc, ExitStack() as ctx:
        persist = ctx.enter_context(tc.tile_pool(name="persist", bufs=1))
        stage_p = ctx.enter_context(tc.tile_pool(name="stage", bufs=2))
        norm_p = ctx.enter_context(tc.tile_pool(name="norms", bufs=2))
        nbg_p = ctx.enter_context(tc.tile_pool(name="nbg", bufs=2))
        work_p = ctx.enter_context(tc.tile_pool(name="work", bufs=4))
        exp_p = ctx.enter_context(tc.tile_pool(name="expb", bufs=8))
        ucst_p = ctx.enter_context(tc.tile_pool(name="ucst", bufs=2))

        z2T = persist.tile([P, N], BF16, tag="z2T")
        fT = persist.tile([P, 2 * FKEYS], BF16, tag="fT")
        z1qT = persist.tile([P, NQ], BF16, tag="z1qT")
        z1qn = persist.tile([P, NQ], BF16, tag="z1qn")
        z1qnf = persist.tile([P, NQ], F32, tag="z1qnf")
        z2qn = persist.tile([P, NQ], F32, tag="z2qn")
        pos_raw = persist.tile([P, QT], F32, tag="pos")
        d_raw = persist.tile([P, QT], F32, tag="draw")
        part = persist.tile([P, QT * NCC], F32, tag="part")
        onesb = persist.tile([P, 32], BF16, tag="onesb")

        def rsqrt_newton(ssq, ntiles):
            """r = 1/sqrt(ssq) elementwise over [P, ntiles]; ACT ln/exp seed
            (stays in the natural_log_exp table set) + one DVE Newton step."""
            r0 = norm_p.tile([P, GROUP], F32, tag="r0")
            t1 = norm_p.tile([P, GROUP], F32, tag="t1")
            nc.scalar.activation(r0[:, :ntiles], ssq[:, :ntiles], AF.Ln)
            nc.scalar.activation(r0[:, :ntiles], r0[:, :ntiles], AF.Exp,
                                 bias=0.0, scale=-0.5)
            nc.vector.tensor_mul(t1[:, :ntiles], r0[:, :ntiles], r0[:, :ntiles])
            nc.vector.tensor_mul(t1[:, :ntiles], t1[:, :ntiles], ssq[:, :ntiles])
            nc.vector.tensor_scalar(
                out=t1[:, :ntiles], in0=t1[:, :ntiles],
                scalar1=-0.5, scalar2=1.5, op0=ALU.mult, op1=ALU.add,
            )
            nc.vector.tensor_mul(r0[:, :ntiles], r0[:, :ntiles], t1[:, :ntiles])
            return r0

        def load_raw(src, row0, ntiles):
            """DMA ntiles row tiles to staging; per-row sum of squares."""
            stage = stage_p.tile([P, GROUP, P], F32, tag="stage")
            nc.sync.dma_start(
                out=stage[:, :ntiles, :],
                in_=src[:, row0:row0 + ntiles * P].rearrange("p (t d) -> p t d", d=P),
            )
            ssq = norm_p.tile([P, GROUP], F32, tag="ssq")
            for t in range(ntiles):
                sq = work_p.tile([P, P], F32, tag="sq")
                nc.vector.scalar_tensor_tensor(
                    out=sq[:, :], in0=stage[:, t, :], scalar=1.0,
                    in1=stage[:, t, :], op0=ALU.bypass, op1=ALU.mult,
                    accum_out=ssq[:, t:t + 1],
                )
            return stage, ssq

        def load_group(src, row0, ntiles):
            stage, ssq = load_raw(src, row0, ntiles)
            return stage, rsqrt_newton(ssq, ntiles)

        def normalize_group(stage, r, ntiles):
            nbg = nbg_p.tile([P, GROUP * P], BF16, tag="nbg")
            for t in range(ntiles):
                nc.vector.tensor_scalar_mul(
                    nbg[:, t * P:(t + 1) * P], stage[:, t, :], r[:, t:t + 1])
            return nbg

        def transpose_group(nbg, dst_T, col0, ntiles):
            dst3 = dst_T[:, col0:col0 + ntiles * P].rearrange(
                "p (t d) -> p t d", d=P)
            nc.sync.dma_start_transpose(dst3, nbg[:, :ntiles * P])

        def finish_keys(stg, ssq, dst_T, col0):
            r = rsqrt_newton(ssq, GROUP)
            nbg = normalize_group(stg, r, GROUP)
            transpose_group(nbg, dst_T, col0, GROUP)

        def stage_keys(src, row0, dst_T, col0):
            stg, ssq = load_raw(src, row0, GROUP)
            finish_keys(stg, ssq, dst_T, col0)

        def deferred_qprep():
            """z2q chain + pos + d: runs in engine slack under early exps."""
            stg, rq = load_group(z2q, 0, QT)
            for t in range(QT):
                nc.vector.tensor_scalar_mul(
                    z2qn[:, t * P:(t + 1) * P], stg[:, t, :], rq[:, t:t + 1])
            for t in range(QT):
                # d_raw[:, t] = sum_d bf16(z1n)^2 (matches the PE diag dot)
                sq = work_p.tile([P, P], F32, tag="dsq")
                nc.gpsimd.tensor_mul(sq[:, :], z1qn[:, t * P:(t + 1) * P],
                                     z1qn[:, t * P:(t + 1) * P])
                nc.vector.tensor_reduce(
                    out=d_raw[:, t:t + 1], in_=sq[:, :], axis=AX.X, op=ALU.add)
                # pos_raw[:, t] = sum_d z1n * z2n (f32)
                mb = work_p.tile([P, P], F32, tag="mb")
                nc.gpsimd.tensor_mul(mb[:, :], z1qnf[:, t * P:(t + 1) * P],
                                     z2qn[:, t * P:(t + 1) * P])
                nc.vector.tensor_reduce(
                    out=pos_raw[:, t:t + 1], in_=mb[:, :], axis=AX.X, op=ALU.add)

        # ---------------- prologue ----------------
        # Critical path to the first exp chunk is ONLY the own-block chain:
        # phase-1 cc0 is the diag chunk, whose keys are z1qT itself.  The
        # z2 g0 DMA+squares start concurrently, but g0's rsqrt Ln is NOT
        # emitted here — it would sit in the in-order ACT FIFO ahead of
        # cc0's already-runnable exps.  It is emitted under cc0 (qt==2).
        stage, ssqQ = load_raw(z1q, 0, QT)
        st0, sq0 = load_raw(z2, 0, GROUP)
        r = rsqrt_newton(ssqQ, QT)
        for t in range(QT):
            nc.vector.tensor_scalar_mul(
                z1qn[:, t * P:(t + 1) * P], stage[:, t, :], r[:, t:t + 1])
        transpose_group(z1qn, z1qT, 0, QT)
        for t in range(QT):
            nc.vector.tensor_scalar_mul(
                z1qnf[:, t * P:(t + 1) * P], stage[:, t, :], r[:, t:t + 1])
        # ones (bf16) for the colsum matmuls
        onesf = persist.tile([P, 32], F32, tag="onesf")
        nc.sync.dma_start(out=onesf[:, :], in_=ones32[:, :])
        nc.vector.tensor_copy(onesb[:, :], onesf[:, :])

        # ---------------- phase 1: diag chunk + z2 keys (FD1=2048) --------
        # cc0 = diag (keys z1qT), cc 1..8 = z2 groups 0..7.  Staging chains
        # for later chunk-columns are emitted a few chunks into each column:
        # their rsqrt Ln/Exp land in the ACT FIFO behind already-runnable
        # exp chunks, so the in-order ACT queue never blocks on a staging
        # dependency (the baseline lost ~40us to this).
        with tc.tile_pool(name="ps1", bufs=2, space="PSUM") as ps1:
            for cc in range(NCC1):
                keysT, koff = (z1qT, 0) if cc == 0 else (z2T, (cc - 1) * FD1)
                for qt in range(QT):
                    if qt == 2 and cc == 0:
                        finish_keys(st0, sq0, z2T, 0)
                    if qt == 6 and cc <= 6:
                        stage_keys(z2, (cc + 1) * GROUP * P, z2T,
                                   (cc + 1) * GROUP * P)
                    if qt == 10 and cc == 0:
                        deferred_qprep()
                    if qt == 12 and 2 <= cc <= 8:
                        g = cc - 2
                        stage_keys(fk, g * GROUP * P, fT, g * GROUP * P)
                    ps = ps1.tile([P, FD1], F32, tag="ps")
                    for j in range(FD1 // SUB):
                        nc.tensor.matmul(
                            ps[:, j * SUB:(j + 1) * SUB],
                            lhsT=z1qT[:, qt * P:(qt + 1) * P],
                            rhs=keysT[:, koff + j * SUB:koff + (j + 1) * SUB],
                            start=True, stop=True,
                        )
                    nc.scalar.activation(
                        ps[:, :], ps[:, :], AF.Exp, bias=0.0, scale=1.0 / TAU,
                        accum_out=part[:, qt * NCC + cc:qt * NCC + cc + 1],
                    )

        # ---------------- phase 2: foreign keys (symmetric off-diag) ------
        # chunk (c2, qh, k): queries = own qtile qh*8+k, keys = fT slice.
        # exp -> SBUF bf16 (rhs for colsum matmuls) + accum_out row sums.
        # colsum matmuls for batch (c2, qh) are emitted interleaved into the
        # NEXT batch's slots (deps already satisfied -> no PE stall), with
        # m=32 ones-lhsT strips accumulating into a 1-bank PSUM tile.
        def emit_colsum(prev, k):
            pcs, pebs, pfd = prev
            for s in range(pfd // SUB):
                nc.tensor.matmul(
                    pcs[32 * s:32 * s + 32, :],
                    lhsT=onesb[:, 0:32],
                    rhs=pebs[k][:, s * SUB:(s + 1) * SUB],
                    start=(k == 0), stop=(k == QT // 2 - 1),
                    tile_position=(0, 32 * s),
                )

        def emit_drain(prev, pqh, pc2):
            pcs, _, pfd = prev
            nst = pfd // SUB
            uc = ucst_p.tile([P, SUB], F32, tag="ucst")
            nc.vector.tensor_copy(uc[:, :], pcs[:, :])
            base = pc2 * FD2
            for s in range(nst):
                nc.sync.dma_start(
                    out=ucols[pqh:pqh + 1, base + s * SUB:base + (s + 1) * SUB],
                    in_=uc[32 * s:32 * s + 1, :],
                )

        with tc.tile_pool(name="ps2", bufs=2, space="PSUM") as ps2, \
                tc.tile_pool(name="cs", bufs=2, space="PSUM") as cs_p:
            prev = None      # (cs_tile, exp_tiles, fd) of previous batch
            prev_loc = None  # (qh, c2)
            for c2 in range(len(CC2)):
                fd = CC2[c2]
                for qh in range(2):
                    cs = cs_p.tile([P, SUB], F32, tag="cs")
                    ebs = []
                    for k in range(QT // 2):
                        qt = qh * (QT // 2) + k
                        koff = qh * FKEYS + c2 * FD2
                        ps = ps2.tile([P, FD2], F32, tag="ps")
                        for j in range(fd // SUB):
                            nc.tensor.matmul(
                                ps[:, j * SUB:(j + 1) * SUB],
                                lhsT=z1qT[:, qt * P:(qt + 1) * P],
                                rhs=fT[:, koff + j * SUB:koff + (j + 1) * SUB],
                                start=True, stop=True,
                            )
                        eb = exp_p.tile([P, FD2], BF16, tag="eb")
                        # previous batch's colsum matmuls: WAR-ordered before
                        # this ACT overwrites the rotated exp buffer
                        if prev is not None:
                            emit_colsum(prev, k)
                        nc.scalar.activation(
                            eb[:, :fd], ps[:, :fd], AF.Exp,
                            bias=0.0, scale=1.0 / TAU,
                            accum_out=part[:, qt * NCC + NCC1 + c2:
                                           qt * NCC + NCC1 + c2 + 1],
                        )
                        ebs.append(eb)
                    if prev is not None:
                        emit_drain(prev, *prev_loc)
                    prev, prev_loc = (cs, ebs, fd), (qh, c2)
            # last batch's colsums + drain (PE/DMA tail under the epilogue)
            for k in range(QT // 2):
                emit_colsum(prev, k)
            emit_drain(prev, *prev_loc)

        # ---------------- epilogue: per-row partial sums ----------------
        S_own = work_p.tile([P, QT], F32, tag="sown")
        for qt in range(QT):
            nc.vector.tensor_reduce(
                out=S_own[:, qt:qt + 1],
                in_=part[:, qt * NCC:(qt + 1) * NCC],
                axis=AX.X, op=ALU.add,
            )
        exp_d = work_p.tile([P, QT], F32, tag="expd")
        nc.scalar.activation(exp_d[:, :], d_raw[:, :], AF.Exp,
                             bias=0.0, scale=1.0 / TAU)
        res = work_p.tile([P, 2 * QT], F32, tag="res")
        nc.vector.tensor_sub(res[:, 0:QT], S_own[:, :], exp_d[:, :])
        nc.vector.tensor_copy(res[:, QT:2 * QT], pos_raw[:, :])
        nc.sync.dma_start(out=out[:, :], in_=res[:, :])

    _split_excess_waits(nc, mybir)
    return nc


def _get_nc():
    if "nc" not in _CACHE:
        _CACHE["nc"] = _build_nc()
    return _CACHE["nc"]


def _foreign_rows(c):
    """Per-core foreign key row indices: [qh, u] -> H rows of z1."""
    rows = []
    for qh in range(2):
        for d in range(1, NCORES):
            p = (c + d) % NCORES
            half = qh if c < p else 1 - qh
            off = p * NQ + half * H
            rows.append(np.arange(off, off + H))
    return np.concatenate(rows)


def kernel(z1, z2):
    from concourse.bass_utils import run_bass_kernel_spmd

    z1 = np.ascontiguousarray(np.asarray(z1, dtype=np.float32))
    z2 = np.ascontiguousarray(np.asarray(z2, dtype=np.float32))
    assert z1.shape == (N, D) and z2.shape == (N, D)

    nc = _get_nc()
    ones = np.ones((P, 32), dtype=np.float32)

    def to_pt(a):
        """[rows, 128] -> [128, rows] tile-transposed: out[p, t*128+d] =
        a[t*128+p, d], matching the kernel's staging layout."""
        T = a.shape[0] // P
        return np.ascontiguousarray(
            a.reshape(T, P, D).transpose(1, 0, 2).reshape(P, T * D))

    z2_pt = to_pt(z2)
    in_maps = [
        {
            "z2": z2_pt,
            "z1q": to_pt(z1[c * NQ:(c + 1) * NQ]),
            "z2q": to_pt(z2[c * NQ:(c + 1) * NQ]),
            "fk": to_pt(z1[_foreign_rows(c)]),
            "ones32": ones,
        }
        for c in range(NCORES)
    ]
    trace = bool(int(os.environ.get("TRNLOSS_TRACE", "0")))
    res = run_bass_kernel_spmd(nc, in_maps, core_ids=list(range(NCORES)), trace=trace)
    if trace:
        _CACHE["exec_time_ns"] = res.exec_time_ns
        print(f"HW exec time: {res.exec_time_ns} ns")

    S = np.zeros(N, dtype=np.float64)
    pos = np.zeros(N, dtype=np.float64)
    for c in range(NCORES):
        o = res.results[c]["out"].astype(np.float64)      # [P, 2*QT]
        S[c * NQ:(c + 1) * NQ] += o[:, 0:QT].T.reshape(-1)
        pos[c * NQ:(c + 1) * NQ] = o[:, QT:2 * QT].T.reshape(-1)
        uc = res.results[c]["ucols"].astype(np.float64)   # [2, FKEYS]
        for qh in range(2):
            for u in range(NCORES - 1):
                d = u + 1
                p = (c + d) % NCORES
                half = qh if c < p else 1 - qh
                off = p * NQ + half * H
                S[off:off + H] += uc[qh, u * H:(u + 1) * H]
    loss = np.mean(np.log(S) - pos / TAU)
    return np.float32(loss)
